# revision 26
# baseline (speedup 1.0000x reference)
"""Trainium2 Bass kernel for nn_ControlModel_g (phi^4 lattice control-variate loss).

Math reformulation (validated to fp32 accuracy against the jax reference):

  The reference evaluates, for each of 16 signed lattice symmetries t and all
  V=256 torus translations s, the tiny MLP g (256->128->1) on the transformed+
  shifted configs, plus its input-gradient at site (0,0), combined with the
  phi^4 force into F[b]; loss = mean((computeO(x) - F - muO)^2).

  1. Symmetry transforms move from x onto W1 (g(T_{-s} R x) = g_R(T_{-s'} x)
     with spatially-transformed weights), so all shifted inputs derive from x
     alone and the force/gradient corrections become fixed permutations.
  2. With b1 == 0 (always true for this model), tanh oddness makes the 8
     sign=-1 transforms algebraically redundant -> half the compute.
  3. The column translation j folds into 16 rotated weight copies
     (W1JBIG[(a,c), (j,r,h)] = W1_r[a, (c-j)%16, h]); the row translation i
     folds into a small shifted-x matrix SH2[(a,c), (i,b)] = x[b,(a+i)%16,c].
     The device work is then one dense matmul Z = SH2^T @ W1JBIG
     (512 x 16384), tanh, and two h-weighted reductions:
         GV = sum_h W2[h] * tanh(Z),   GD = sum_h (W2*W1[0])[h] * tanh(Z)^2
  4. Sharding: data-parallel over the j columns - core k takes j in {2k,2k+1}
     (2048 of the 16384 output columns). No collectives needed; the final
     O(B*V) combine (force permutations, computeO, loss) is host-side numpy.

Device pipeline (per core), tuned from CoreSim cost-model traces + HW timing:

  - All operands bf16 (validated: total loss rel err ~1.2e-3 vs the 2e-2
    gate); PSUM stays f32 (TRN2 matmul cannot write bf16 PSUM).
  - Work is cut into 8 pipeline units of 2 h-groups (1024 Z-columns) each.
    Per unit: 4 accumulating Z-matmuls (K=256 via 2 passes) -> one f32 PSUM
    tile (2 banks), one 1024-wide tanh on ScalarE -> bf16 SBUF, square on
    VectorE, then 4 CONCURRENT col-tiled (tile_position) reduce matmuls with
    hi/lo-split bf16 weights into a single PSUM bank, evacuated by a
    VectorE f32->bf16 copy and DMA'd out (pairs of units share one DMA).
  - Emission is explicitly software-pipelined: PE's in-order stream is
    [Z0 Z1 r0 Z2 r1 ... Z7 r6 r7] (reduces lag Z by LAG_R=2 units,
    evacuation 2 more) so reduce matmuls waiting on tanh/square never
    head-of-line-block the next unit's Z work. Without this the whole
    machine serializes to the sum of all engine busies (measured 37us).
  - DMA: weight chunks and SH2 halves are spread across the three rings
    (sync/scalar HWDGE + gpsimd SWDGE) so the first Z-matmul unblocks at
    ~1.5us and later chunks stream under compute. Output is the compact
    hi/lo rows only (0.5 MB total vs 2 MB full-tile dump).
  - A dummy activation pulls the ~2.7us ACT table load, and a few dummy
    matmuls warm the PE HAM clock gate, both under the initial DMA wait.
"""

import numpy as np
import ml_dtypes

L = 16
Y = 4
KAPPA = 0.25
LAM = 0.5
B = 32
V = L * L          # 256
H = 128
NCORES = 8
JPER = L // NCORES         # j values per core = 2
NCOLS = JPER * 8 * H       # 2048 columns per core (j, r, h)
M = L * B                  # 512 rows (i, b)
NSET = 4                   # device processes 4 groups per set
NGRP = JPER * 8            # 16 reduce groups per core

# ---------------------------------------------------------------------------
# host-side lattice helpers
# ---------------------------------------------------------------------------

def _force(phi):
    nbr = (np.roll(phi, 1, 1) + np.roll(phi, -1, 1)
           + np.roll(phi, 1, 2) + np.roll(phi, -1, 2))
    return 2.0 * KAPPA * nbr - 2.0 * phi - 4.0 * LAM * phi * (phi * phi - 1.0)


def _computeO(x):
    x0 = x.mean(axis=1)
    x0 = x0 - x0.mean(axis=0, keepdims=True)
    return (x0 * np.roll(x0, -Y, axis=1)).mean(axis=1)


def _spatial_ops():
    ops = []
    for k in range(4):
        ops.append(lambda y, k=k: np.rot90(y, k=k, axes=(0, 1)))
        ops.append(lambda y, k=k: np.flip(np.rot90(y, k=k, axes=(0, 1)), axis=0))
    return ops


def _op_tables():
    """Per spatial op r: inverse site permutation (for W1) and the force
    permutation mu_r[s] = pi_r(rho_r^{-1}(s))."""
    ops = _spatial_ops()
    IDX = np.arange(V).reshape(L, L)
    inv_perms, mus = [], []
    for op in ops:
        pi = op(IDX).reshape(-1)
        inv = np.empty(V, np.int64)
        inv[pi] = np.arange(V)
        inv_perms.append(inv)
        rho = np.empty(V, np.int64)
        opIDX = op(IDX)
        for i in range(L):
            for j in range(L):
                shifted = np.roll(np.roll(opIDX, -i, 0), -j, 1).reshape(V)
                rho[i * L + j] = shifted[inv][0]
        rho_inv = np.empty(V, np.int64)
        rho_inv[rho] = np.arange(V)
        mus.append(pi[rho_inv])
    return inv_perms, mus


_TABLES = None

def _tables():
    global _TABLES
    if _TABLES is None:
        _TABLES = _op_tables()
    return _TABLES


# ---------------------------------------------------------------------------
# device program (built once, cached)
# ---------------------------------------------------------------------------

_PROG = None

def _build_program(reps=None, dma_in_loop=False, level=5, opts=None):
    opts = dict(opts or {})
    warm_n = opts.get("warm_n", 4)
    lag_r = opts.get("lag_r", 2)
    lag_e = opts.get("lag_e", 2)
    rings = opts.get("rings", "sggg")
    nbufs = opts.get("nbufs", (4, 5, 5, 8))
    out_alt = opts.get("out_alt", False)
    out_pair = opts.get("out_pair", True)
    import concourse.bass as bass
    import concourse.tile as tile
    from concourse import bacc, mybir

    f32 = mybir.dt.float32
    f32r = mybir.dt.float32r
    bf16 = mybir.dt.bfloat16
    MUL = mybir.AluOpType.mult
    TANH = mybir.ActivationFunctionType.Tanh

    nc = bacc.Bacc("TRN2", target_bir_lowering=False, debug=False,
                   num_devices=NCORES)

    sh_d = nc.dram_tensor("sh", (128, 2 * M), bf16, kind="ExternalInput")
    w1_d = nc.dram_tensor("w1j", (NSET, 128, 1024), bf16, kind="ExternalInput")
    rw_d = nc.dram_tensor("rw", (128, 64), bf16, kind="ExternalInput")
    # out[zu, 32*q + u, m]: zu = 2*s + zp covers groups {2zu, 2zu+1};
    # col slot q: 0,1 = gv of the two groups, 2,3 = gd; u = hi/lo rows
    out_d = nc.dram_tensor("gvgd", (2 * NSET, 128, M), bf16,
                           kind="ExternalOutput")

    NU = 2 * NSET          # 8 pipeline units of 2 h-groups each
    LAG_R = lag_r          # reduce matmuls lag Z-matmuls by this many units
    LAG_E = lag_e          # evacuation lags reduces by this many more

    with tile.TileContext(nc) as tc:
        with (
            tc.tile_pool(name="consts", bufs=2 if dma_in_loop else 1) as cpool,
            tc.tile_pool(name="warm", bufs=1) as mpool,
            tc.tile_pool(name="wpsum", bufs=1, space=bass.MemorySpace.PSUM) as wppool,
            tc.tile_pool(name="wp", bufs=nbufs[0]) as wpool,
            tc.tile_pool(name="zp", bufs=2, space=bass.MemorySpace.PSUM) as zpool,
            tc.tile_pool(name="rp", bufs=3, space=bass.MemorySpace.PSUM) as rpool,
            tc.tile_pool(name="atp", bufs=nbufs[1]) as apool,
            tc.tile_pool(name="sqp", bufs=nbufs[2]) as qpool,
            tc.tile_pool(name="outp", bufs=nbufs[3]) as opool,
        ):
            def load_consts():
                rw_t = cpool.tile([128, 64], bf16, tag="rw")
                nc.scalar.dma_start(rw_t[:], rw_d[:])
                shc = cpool.tile([128, 2 * M], bf16, tag="sh")
                # halves split across rings; k=0 (behind wt0 on sync) and k=1
                # (on scalar) land by ~1.5us so the first Z pair starts early
                nc.sync.dma_start(shc[:, 0:M], sh_d[:, 0:M])
                nc.scalar.dma_start(shc[:, M:2 * M], sh_d[:, M:2 * M])
                return rw_t, shc

            if not dma_in_loop:
                consts = load_consts()

            # one-time warmup: pull the ACT table load and the PE HAM
            # un-throttle under the initial DMA wait
            dum = mpool.tile([128, 256], bf16, tag="dum")
            nc.vector.memset(dum[:], 0.0)
            dumo = mpool.tile([128, 16], bf16, tag="dumo")
            nc.scalar.activation(dumo[:], dum[:, 0:16], TANH)
            rps_w = wppool.tile([128, M], f32, tag="rpsw")
            for _ in range(warm_n):
                nc.tensor.matmul(rps_w[:, 0:256], dum[:, 0:128],
                                 dum[:], start=True, stop=True)

            # weight chunk s -> DMA ring, spread so transfers run in parallel
            ringmap = {"s": nc.sync, "g": nc.gpsimd, "a": nc.scalar}
            w_rings = [ringmap[c] for c in rings]

            def body():
                if dma_in_loop:
                    rw_t, shc = load_consts()
                else:
                    rw_t, shc = consts
                wts, ats, asqs, rpss = [], {}, {}, {}
                pair_tiles = {}
                for s in range(NSET):
                    wt = wpool.tile([128, 1024], bf16, tag="w")
                    wts.append(wt)
                # wt0 FIRST on sync (ahead of shc k0) so the earliest Z
                # matmuls unblock ASAP; later chunks stream via SWDGE
                w_rings[0].dma_start(wts[0][:], w1_d[0])
                for s in range(1, NSET):
                    w_rings[s].dma_start(wts[s][:], w1_d[s])
                # software-pipelined emission: PE's in-order stream is
                # [Z0 Z1 r0 Z2 r1 ... Z7 r6 r7] so the reduce matmuls never
                # head-of-line-block the next unit's Z work while ACT/DVE
                # produce tanh/square.
                for t in range(NU + LAG_R + LAG_E):
                    zu = t
                    if zu < NU and level >= 1:
                        s, zp = divmod(zu, 2)
                        zt = zpool.tile([128, 2 * M], f32, tag="zt")
                        for gh in range(2):
                            g = 2 * zp + gh
                            hs = slice(gh * M, (gh + 1) * M)
                            for k in range(2):
                                nc.tensor.matmul(
                                    zt[:, hs],
                                    wts[s][:, k * 512 + g * H:
                                           k * 512 + (g + 1) * H],
                                    shc[:, k * M:(k + 1) * M],
                                    start=(k == 0), stop=(k == 1))
                        if level >= 2:
                            at = apool.tile([128, 2 * M], bf16, tag="at")
                            nc.scalar.activation(at[:], zt[:], TANH)
                            ats[zu] = at
                        if level >= 3:
                            asq = qpool.tile([128, 2 * M], bf16, tag="asq")
                            nc.vector.tensor_tensor(asq[:], ats[zu][:],
                                                    ats[zu][:], MUL)
                            asqs[zu] = asq
                    ru = t - LAG_R
                    if 0 <= ru < NU and level >= 3:
                        # 4 concurrent col-tiled reduce matmuls -> one bank
                        rps = rpool.tile([128, M], f32, tag="rps")
                        for go in range(2):
                            nc.tensor.matmul(
                                rps[32 * go:32 * (go + 1), :], rw_t[:, 0:32],
                                ats[ru][:, go * M:(go + 1) * M], start=True,
                                stop=True, tile_position=(0, 32 * go))
                        for go in range(2):
                            nc.tensor.matmul(
                                rps[64 + 32 * go:96 + 32 * go, :],
                                rw_t[:, 32:64],
                                asqs[ru][:, go * M:(go + 1) * M], start=True,
                                stop=True, tile_position=(0, 64 + 32 * go))
                        rpss[ru] = rps
                    eu = t - LAG_R - LAG_E
                    if 0 <= eu < NU and level >= 4:
                        if not out_pair:
                            rsb = opool.tile([128, M], bf16, tag="rsb")
                            nc.vector.tensor_copy(rsb[:], rpss[eu][:])
                            if level >= 5:
                                ring = (nc.scalar if (out_alt and eu % 2)
                                        else nc.sync)
                                ring.dma_start(out_d[eu], rsb[:])
                        else:
                            if eu % 2 == 0:
                                rsbp = opool.tile([128, 2 * M], bf16,
                                                  tag="rsb")
                                pair_tiles[eu // 2] = rsbp
                            rsbp = pair_tiles[eu // 2]
                            nc.vector.tensor_copy(
                                rsbp[:, (eu % 2) * M:(eu % 2 + 1) * M],
                                rpss[eu][:])
                            if eu % 2 == 1 and level >= 5:
                                nc.sync.dma_start(
                                    out_d[eu - 1:eu + 1].rearrange(
                                        "zh p m -> p zh m"),
                                    rsbp[:].rearrange(
                                        "p (zh m) -> p zh m", zh=2))

            if reps is None:
                body()
            else:
                with tc.For_i(0, reps, 1):
                    body()

    nc.compile()
    return nc


def _get_program():
    global _PROG
    if _PROG is None:
        _PROG = _build_program()
    return _PROG


# ---------------------------------------------------------------------------
# numpy fallback (general b1; never hit for this model's inputs)
# ---------------------------------------------------------------------------

def _numpy_reference(x, W1, b1, W2, b2, muO):
    def transforms(x):
        outs = []
        for sign in (1.0, -1.0):
            sx = sign * x
            for k in range(4):
                rx = np.rot90(sx, k=k, axes=(1, 2))
                outs.append(rx)
                outs.append(np.flip(rx, axis=1))
        return np.stack(outs)

    idx = (np.arange(L)[:, None] + np.arange(L)[None, :]) % L
    Ftot = np.zeros(B, np.float32)
    for tx in transforms(x):
        fx = _force(tx).reshape(B, V)
        sh = tx[:, idx, :][:, :, :, idx]
        shifts = np.transpose(sh, (1, 3, 0, 2, 4)).reshape(V, B, V)
        z = shifts @ W1 + b1
        h = np.tanh(z)
        gvals = h @ W2 + b2[0]
        grads = ((1.0 - h * h) * W2) @ W1[0]
        Ftot += (grads + gvals * fx.T).sum(axis=0)
    F = Ftot / 16.0
    delta = _computeO(x) - F
    return np.float32(((delta - muO[0]) ** 2).mean())


# ---------------------------------------------------------------------------
# entry point
# ---------------------------------------------------------------------------

def _hilo(w):
    hi = w.astype(ml_dtypes.bfloat16)
    lo = (w - hi.astype(np.float32)).astype(ml_dtypes.bfloat16)
    out = np.zeros((128, 32), ml_dtypes.bfloat16)
    out[:, 0] = hi
    out[:, 1] = lo
    return out


def kernel(x, W1, b1, W2, b2, muO):
    x = np.asarray(x, np.float32)
    W1 = np.asarray(W1, np.float32)
    b1 = np.asarray(b1, np.float32)
    W2 = np.asarray(W2, np.float32)
    b2 = np.asarray(b2, np.float32)
    muO = np.asarray(muO, np.float32)

    if np.any(b1 != 0.0):
        return _numpy_reference(x, W1, b1, W2, b2, muO)

    inv_perms, mus = _tables()
    W1flat = W1.reshape(V, H)

    # SH2[(a,c), (i,b)] = x[b, (a+i)%L, c]; device layout [p, (k, m)]
    SH2 = np.empty((V, M), np.float32)
    for i in range(L):
        SH2[:, i * B:(i + 1) * B] = np.roll(x, -i, axis=1).reshape(B, V).T
    sh_in = np.ascontiguousarray(
        SH2.reshape(2, 128, M).transpose(1, 0, 2).reshape(128, 2 * M)
        .astype(ml_dtypes.bfloat16))

    # W1JBIG columns (j, r, h); per-core slice j in {2k, 2k+1}, packed as
    # [set, p, (k, g, h)] with group G = jl*8 + r = 4*set + g
    W1r_imgs = [W1flat[inv].reshape(L, L, H) for inv in inv_perms]
    w1_cores = []
    for k in range(NCORES):
        blk = np.empty((V, JPER, 8, H), np.float32)
        for jl in range(JPER):
            j = JPER * k + jl
            for r in range(8):
                blk[:, jl, r, :] = np.roll(W1r_imgs[r], j, axis=1).reshape(V, H)
        full = blk.reshape(2, 128, NSET, 4, H)        # [k, p, set, g, h]
        w1_cores.append(np.ascontiguousarray(
            full.transpose(2, 1, 0, 3, 4).reshape(NSET, 128, 1024)
            .astype(ml_dtypes.bfloat16)))

    CW = (W1flat[0] * W2).astype(np.float32)
    rw_in = np.ascontiguousarray(
        np.concatenate([_hilo(W2), _hilo(CW)], axis=1))

    nc = _get_program()
    from concourse import bass_utils
    in_maps = [{"sh": sh_in, "w1j": w1_cores[k], "rw": rw_in}
               for k in range(NCORES)]
    res = bass_utils.run_bass_kernel_spmd(nc, in_maps,
                                          core_ids=list(range(NCORES)))

    # assemble GV[i,b,j,r], GD[i,b,j,r] from per-core (4, 2, 4096) outputs
    GV = np.empty((L, B, L, 8), np.float32)
    GD = np.empty((L, B, L, 8), np.float32)
    for k in range(NCORES):
        arr = np.asarray(res.results[k]["gvgd"], np.float32)
        t = arr.reshape(2 * NSET, 4, 32, M)[:, :, 0:2]  # [zu, q, u, m]
        t = t.sum(axis=2)                               # hi+lo -> [zu, q, m]
        gvf = t[:, 0:2, :].reshape(NGRP, L, B)          # G = 2*zu + go
        gdf = t[:, 2:4, :].reshape(NGRP, L, B)
        per = gvf.reshape(JPER, 8, L, B)                # [jl, r, i, b]
        perd = gdf.reshape(JPER, 8, L, B)
        GV[:, :, JPER * k:JPER * (k + 1), :] = per.transpose(2, 3, 0, 1)
        GD[:, :, JPER * k:JPER * (k + 1), :] = perd.transpose(2, 3, 0, 1)

    fxo = _force(x).reshape(B, V)
    Csum = float(CW.sum())
    Ftot = np.zeros(B, np.float64)
    for r in range(8):
        gval = GV[:, :, :, r].transpose(0, 2, 1).reshape(V, B)
        gdot = Csum - GD[:, :, :, r].transpose(0, 2, 1).reshape(V, B)
        fxt = fxo[:, mus[r]].T
        Ftot += (gdot + gval * fxt).sum(axis=0)
    F = (Ftot / 8.0).astype(np.float32)

    delta = _computeO(x) - F
    return np.float32(((delta - muO[0]) ** 2).mean())
